# revision 1
# baseline (speedup 1.0000x reference)
"""Trainium2 Bass kernel for causal multi-head self-attention with RoPE.

Problem: x[4,2048,1024], 16 heads, head_dim 64, causal, RoPE theta=1e4,
qkv proj + out proj.  Sharded over 8 cores: core c -> batch c//2, head
group c%2 (8 heads).  Host sums the two head-group partial outputs per
batch (the w_out all-reduce).

v2 design (vs v1): all operands bf16 (f32 PSUM accumulation), one
global software-pipelined instruction stream so the PE never idles:
  - phase 1 half-0 QKV (bf16) + RoPE half-0 (DVE bf16 2x/4x modes)
  - qc0 attention units interleaved into half-1 QKV matmul groups
  - qc1..3 attention with 2-unit score->PV lookahead, out-projection of
    the previous chunk as PE filler, diag tiles shrunk to [rel:512]
  - softmax denominator via ones-column in V; normalize via DVE fast
    reciprocal + gpsimd partition_broadcast (no DRAM round trip)
"""
import numpy as np
import ml_dtypes

import concourse.bass as bass
import concourse.bacc as bacc
import concourse.mybir as mybir
import concourse.tile as tile

F32 = mybir.dt.float32
BF16 = mybir.dt.bfloat16
AF = mybir.ActivationFunctionType

THETA = 10000.0
S = 2048
D = 1024
NH = 8          # heads per core
DH = 64
EL = 512        # local head dims (NH*DH)
HALF = 1024     # tokens per QKV phase-1 half
PI_2 = 1.5707963267948966

SHUF_MASK = [(i + 16) % 32 for i in range(32)]

DEBUG = False


def build_nc():
    nc = bacc.Bacc("TRN2", target_bir_lowering=False, debug=False)

    xT = nc.dram_tensor("xT", [D, S], BF16, kind="ExternalInput").ap()
    wqkT = nc.dram_tensor("wqkT", [D, 2 * EL], BF16, kind="ExternalInput").ap()
    wvT = nc.dram_tensor("wvT", [D, EL], BF16, kind="ExternalInput").ap()
    wout = nc.dram_tensor("wout", [EL, D], BF16, kind="ExternalInput").ap()
    posf = nc.dram_tensor("posf", [1, S], F32, kind="ExternalInput").ap()
    invf = nc.dram_tensor("invf", [128, 1], F32, kind="ExternalInput").ap()
    sgn = nc.dram_tensor("sgn", [128, 1], F32, kind="ExternalInput").ap()
    shufP = nc.dram_tensor("shufP", [128, 128], BF16, kind="ExternalInput").ap()
    y = nc.dram_tensor("y", [S, D], BF16, kind="ExternalOutput").ap()

    dbg = None
    if DEBUG:
        dbg = {
            "qk0": nc.dram_tensor("dbg_qk0", [128, S], BF16, kind="ExternalOutput").ap(),
            "qk4": nc.dram_tensor("dbg_qk4", [128, S], BF16, kind="ExternalOutput").ap(),
            "v0": nc.dram_tensor("dbg_v0", [128, NH * 65], BF16, kind="ExternalOutput").ap(),
            "po00": nc.dram_tensor("dbg_po00", [65, 512], F32, kind="ExternalOutput").ap(),
            "pr00": nc.dram_tensor("dbg_pr00", [128, 2, 512], BF16, kind="ExternalOutput").ap(),
            "oc00": nc.dram_tensor("dbg_oc00", [64, 512], BF16, kind="ExternalOutput").ap(),
            "rec_row": nc.dram_tensor("dbg_rec_row", [1, 512], F32, kind="ExternalOutput").ap(),
            "rec0": nc.dram_tensor("dbg_rec0", [1, 512], F32, kind="ExternalOutput").ap(),
            "bca": nc.dram_tensor("dbg_bca", [64, 512], F32, kind="ExternalOutput").ap(),
        }

    with tile.TileContext(nc) as tc:
        kernel_body(tc, xT, wqkT, wvT, wout, posf, invf, sgn, shufP, y, dbg)
    nc.compile()
    return nc


def kernel_body(tc, xT, wqkT, wvT, wout, posf, invf, sgn, shufP, y, dbg=None):
    nc = tc.nc
    with (
        tc.tile_pool(name="sb", bufs=1) as sb,
        tc.tile_pool(name="pp", bufs=1, space="PSUM") as pp,
    ):
        _kernel(tc, sb, pp, xT, wqkT, wvT, wout, posf, invf, sgn, shufP, y, dbg)


def _kernel(tc, sb, pp, xT, wqkT, wvT, wout, posf, invf, sgn, shufP, y, dbg=None):
    nc = tc.nc

    def dump(name, ap):
        if dbg is None or name not in dbg:
            return
        if ap.space == bass.MemorySpace.PSUM:
            tmp = sb.tile(list(ap.shape), F32, tag=f"dbg_{name}", bufs=1)
            nc.vector.tensor_copy(tmp, ap)
            ap = tmp
        nc.sync.dma_start(out=dbg[name], in_=ap)

    # ---- persistent tiles ----------------------------------------------
    qk_sb = [sb.tile([128, S], BF16, tag="qk", bufs=8, name=f"qk{t}")
             for t in range(8)]
    vtiles = [sb.tile([128, NH, 65], BF16, tag="v", bufs=16, name=f"v{t}")
              for t in range(16)]
    mtri = sb.tile([128, 128], BF16, tag="mtri", bufs=1, name="mtri")
    ctab = sb.tile([128, S], BF16, tag="cs", bufs=2, name="ctab")
    stab = sb.tile([128, S], BF16, tag="cs", bufs=2, name="stab")

    # ---- input DMAs (ordered for earliest PE start) --------------------
    xh = {}
    for H in (0, 1):
        xh[H] = [sb.tile([128, HALF], BF16, tag="xh", bufs=12, name=f"x{H}{d}")
                 for d in range(8)]

    def dma_xh(H):
        # split across two DGE queues so the startup stream isn't serialized;
        # 512-column halves so the first sc-sweep starts after ~128KB
        s0 = H * HALF
        for sc in range(2):
            for d in range(8):
                eng = nc.sync if d % 2 == 0 else nc.gpsimd
                c0 = s0 + sc * 512
                eng.dma_start(out=xh[H][d][:, sc * 512:(sc + 1) * 512],
                              in_=xT[d * 128:(d + 1) * 128, c0:c0 + 512])

    def dma_wqk(blk, ep, eng=None):
        wcol = blk * EL + ep * 256
        wts = []
        for d in range(8):
            w_t = sb.tile([128, 256], BF16, tag="w", bufs=12, name="wqk")
            (eng or nc.scalar).dma_start(
                out=w_t, in_=wqkT[d * 128:(d + 1) * 128, wcol:wcol + 256])
            wts.append(w_t)
        return wts

    wts00 = dma_wqk(0, 0)   # first q/k weight group before x: PE starts sooner
    wts01 = dma_wqk(0, 1)   # second group before wv (needed ~8us earlier)
    dma_xh(0)

    # small table loads AFTER the critical x stream (~650ns queue cost each;
    # tables aren't consumed until ~10us in)
    invf_sb = sb.tile([128, 1], F32, tag="cvec", bufs=2)
    nc.sync.dma_start(out=invf_sb, in_=invf)
    sgn_sb = sb.tile([128, 1], F32, tag="cvec", bufs=2)
    nc.sync.dma_start(out=sgn_sb, in_=sgn)
    shufP_sb = sb.tile([128, 128], BF16, tag="shufP", bufs=1)
    nc.sync.dma_start(out=shufP_sb, in_=shufP)
    posf_sb = sb.tile([1, S], F32, tag="rt", bufs=2)
    nc.sync.dma_start(out=posf_sb, in_=posf)

    wv_sb = []
    for d in range(8):
        wv_t = sb.tile([128, EL], BF16, tag="wv", bufs=8, name=f"wv{d}")
        nc.scalar.dma_start(out=wv_t, in_=wvT[d * 128:(d + 1) * 128, :])
        wv_sb.append(wv_t)

    # ---- phase 0: RoPE tables + causal triangle mask -------------------
    nc.vector.memset(mtri, 1.0)
    nc.gpsimd.affine_select(out=mtri, in_=mtri,
                            compare_op=mybir.AluOpType.is_ge, fill=0.0,
                            base=0, channel_multiplier=-1, pattern=[[1, 128]])

    pos_b = sb.tile([128, S], F32, tag="rt", bufs=2)
    nc.gpsimd.partition_broadcast(pos_b, posf_sb)
    angles = sb.tile([128, S], F32, tag="rt", bufs=2)
    nc.vector.tensor_scalar_mul(angles, pos_b, invf_sb)
    # range-reduce angles into [-pi, pi]:  k = round(angle / 2pi) via the
    # magic-constant trick, then 3-term Cody-Waite  x - k*2pi.
    TWO_PI = 6.283185307179586
    MAGIC = 1.5 * 2.0 ** 23
    kq = sb.tile([128, S], F32, tag="rt", bufs=2)
    nc.vector.tensor_scalar_mul(kq, angles, 1.0 / TWO_PI)
    nc.vector.tensor_scalar(kq, kq, MAGIC, MAGIC,
                            mybir.AluOpType.add, mybir.AluOpType.subtract)
    CW1 = 6.28125
    CW2 = float(np.float32(TWO_PI - CW1))
    CW3 = float(TWO_PI - CW1 - np.float64(np.float32(TWO_PI - CW1)))
    nc.vector.cody_waite_cascade(angles, angles, kq, CW1, CW2, CW3)
    nc.vector.add_range_wrap(kq, angles, 0.0, np.pi, TWO_PI)
    nc.scalar.activation(stab, kq, AF.Sin)
    nc.vector.add_range_wrap(angles, angles, PI_2, np.pi, TWO_PI)
    nc.scalar.activation(ctab, angles, AF.Sin)
    nc.vector.tensor_scalar_mul(stab, stab, sgn_sb)

    # ---- phase 1 building blocks ---------------------------------------
    def qk_group(H, blk, ep, wts=None):
        s0 = H * HALF
        if wts is None:
            wts = dma_wqk(blk, ep)
        pss = [pp.tile([128, 2, 512], F32, tag="ps", bufs=3, name="psqk")
               for _ in range(2)]
        # sc-major then d-major: each matmul only needs one 512-col half of
        # one xh tile, so the PE streams right behind the x DMA
        for sc in range(2):
            for d in range(8):
                for ei in range(2):
                    nc.tensor.matmul(
                        pss[ei][:, sc, :], lhsT=wts[d][:, ei * 128:(ei + 1) * 128],
                        rhs=xh[H][d][:, sc * 512:(sc + 1) * 512],
                        start=(d == 0), stop=(d == 7))
        closures = []
        for ei in range(2):
            ps = pss[ei]
            t = blk * 4 + ep * 2 + ei
            sl = qk_sb[t][:, s0:s0 + HALF]
            # alternate the PSUM->SBUF copy between ACT and DVE to balance
            # the two queues (both are ~60-80% busy in phase 1)
            if ei == 0:
                nc.scalar.copy(sl, ps.rearrange("p a b -> p (a b)"))
            else:
                nc.vector.tensor_copy(sl, ps.rearrange("p a b -> p (a b)"))
            # RoPE partner values via a PE permutation matmul (stream_shuffle
            # is broken for bf16 and would couple the PE to the DVE through
            # the ps tiles; a 128x128 permutation matmul costs ~0.5us PE).
            # The perm matmul's only data dep is the sl copy above, so write
            # it back IN PLACE into this group's own ps tile (its qkv data is
            # dead once the copy drains) - zero extra PSUM allocations.
            shufB = sb.tile([128, HALF], BF16, tag="rsh", bufs=2, name="shufB")
            for sc in range(2):
                nc.tensor.matmul(ps[:, sc, :], lhsT=shufP_sb,
                                 rhs=sl[:, sc * 512:(sc + 1) * 512],
                                 start=True, stop=True)
            if ei == 0:
                nc.vector.tensor_copy(shufB, ps.rearrange("p a b -> p (a b)"))
            else:
                nc.scalar.copy(shufB, ps.rearrange("p a b -> p (a b)"))

            def fin(sl=sl, shufB=shufB, s0=s0):
                t1 = sb.tile([128, HALF], BF16, tag="rt1", bufs=2, name="t1")
                nc.vector.tensor_mul(t1, sl, ctab[:, s0:s0 + HALF])
                nc.vector.tensor_mul(shufB, shufB, stab[:, s0:s0 + HALF])
                nc.vector.tensor_add(sl, t1, shufB)
            closures.append(fin)
        return closures

    def v_pair(H, p):
        # half-0: use the attention-idle po tag so the ps rotation serves
        # only the qkv chains (same decoupling as the perm matmuls)
        if H == 0:
            pss = [pp.tile([128, 512], F32, tag="po", bufs=2, name="psv")
                   for _ in range(2)]
        else:
            ps = pp.tile([128, 2, 512], F32, tag="ps", bufs=3, name="psv")
            pss = [ps[:, 0, :], ps[:, 1, :]]
        for d in range(8):
            for i in range(2):
                st = 2 * p + i
                nc.tensor.matmul(
                    pss[i], lhsT=xh[H][d][:, st * 128:(st + 1) * 128],
                    rhs=wv_sb[d], start=(d == 0), stop=(d == 7))
        for i in range(2):
            st = 2 * p + i
            vt = vtiles[H * 8 + st]
            if i == 0:
                nc.scalar.copy(vt[:, :, 0:64],
                               pss[i].rearrange("p (h e) -> p h e", h=NH))
            else:
                nc.vector.tensor_copy(vt[:, :, 0:64],
                                      pss[i].rearrange("p (h e) -> p h e", h=NH))
            # ones column via ACT: out = Copy(in*0 + 1) -> softmax denominator
            nc.scalar.activation(vt[:, :, 64:65], pss[i][:, 0:NH], AF.Copy,
                                 bias=1.0, scale=0.0)

    # ---- phase 1, half 0 (straight line, RoPE finished inline) ---------
    pre = {(0, 0): wts00, (0, 1): wts01}
    for blk in range(2):
        for ep in range(2):
            fins = qk_group(0, blk, ep, wts=pre.get((blk, ep)))
            v_pair(0, blk * 2 + ep)
            for fin in fins:
                fin()

    dma_xh(1)
    wout_sb = []
    for pair in range(NH // 2):
        wo_t = sb.tile([128, D], BF16, tag="wout", bufs=4, name=f"wo{pair}")
        nc.scalar.dma_start(out=wo_t, in_=wout[pair * 128:(pair + 1) * 128, :])
        wout_sb.append(wo_t)

    # ---- phase 2: attention units with global software pipeline --------
    # Units per head: diag pairs FIRST so their masks (DVE) enjoy the full
    # S->PV lookahead slack, then mask-free full tiles.
    units = []
    for qc in range(4):
        nd = 4 * qc
        for j in range(NH):
            for p in range(2):
                units.append(dict(kind='d', qc=qc, j=j, kis=(nd + 2 * p, nd + 2 * p + 1)))
            for g0 in range(0, nd, 2):
                units.append(dict(kind='g', qc=qc, j=j, kis=(g0, g0 + 1)))
            units[-1]['last'] = True

    # fillers before S-emission of unit index i
    fillers = {}
    rope_fins = []

    def run_qk_group1(blk, ep):
        rope_fins.extend(qk_group(1, blk, ep))

    seq = []
    for blk in range(2):
        for ep in range(2):
            seq.append(lambda blk=blk, ep=ep: run_qk_group1(blk, ep))
            seq.append(lambda p=blk * 2 + ep: v_pair(1, p))
    for i, f in enumerate(seq):
        fillers.setdefault(2 * i, []).append(f)

    po_map = {}
    ocs = {}

    def emit_S(u):
        qc, j = u['qc'], u['j']
        jt, jb = j // 2, (j % 2) * 64
        q0 = qc * 512
        qh = qk_sb[jt][jb:jb + 64, q0:q0 + 512]
        kt = qk_sb[4 + jt]
        ps = pp.tile([128, 2, 512], F32, tag="ps", bufs=3, name="ps")
        pr = sb.tile([128, 2, 512], BF16, tag="pr", bufs=8, name="pr")
        u['pr'] = pr
        rels = []
        for i, ki in enumerate(u['kis']):
            rel = ki * 128 - q0 if u['kind'] == 'd' else 0
            rels.append(rel)
            nc.tensor.matmul(
                ps[:, i, rel:512],
                lhsT=kt[jb:jb + 64, ki * 128:(ki + 1) * 128],
                rhs=qh[:, rel:512], start=True, stop=True)
        u['rels'] = rels
        if u['kind'] == 'g':
            nc.scalar.activation(pr, ps, AF.Exp, scale=0.125)
        else:
            # One batched exp per diag pair over [min_rel:512]; the exp of
            # the not-yet-needed columns of the second tile lands on garbage
            # PSUM but is never read (PV + mask only touch [rel:512]).
            mrel = min(rels)
            nc.scalar.activation(pr[:, :, mrel:512], ps[:, :, mrel:512],
                                 AF.Exp, scale=0.125)
            for i, rel in enumerate(rels):
                blk_ap = pr[:, i, rel:rel + 128]
                nc.vector.tensor_mul(blk_ap, blk_ap, mtri)

    def emit_PV(u):
        qc, j = u['qc'], u['j']
        key = (qc, j)
        if key not in po_map:
            po_map[key] = pp.tile([65, 512], F32, tag="po", bufs=2, name="po")
        po = po_map[key]
        nd = 4 * qc
        stop_ki = nd - 1 if nd > 0 else nd + 3   # last unit is G for qc>0
        pr = u['pr']
        for i, ki in enumerate(u['kis']):
            rel = u['rels'][i]
            nc.tensor.matmul(
                po[:, rel:512], lhsT=vtiles[ki][:, j, :],
                rhs=pr[:, i, rel:512],
                start=(ki == nd), stop=(ki == stop_ki))
        if (qc, j) == (0, 0):
            dump("pr00", pr)
            if u.get('last'):
                dump("po00", po)

    def normalize(qc, j):
        po = po_map.pop((qc, j))
        rec65 = sb.tile([65, 512], F32, tag="rec", bufs=2, name="rec")
        # custom-DVE ops read zeros from PSUM and misbehave off partition 0:
        # stage den PSUM->SBUF (aligned), DMA it to partition 0, recip there.
        nc.vector.tensor_copy(rec65[64:65, :], po[64:65, :])
        rec0a = sb.tile([1, 512], F32, tag="rec0a", bufs=2, name="rec0a")
        nc.sync.dma_start(out=rec0a, in_=rec65[64:65, :])
        rec0 = sb.tile([1, 512], F32, tag="rec0", bufs=2, name="rec0")
        nc.vector.reciprocal_approx_fast(rec0, rec0a)
        bca = sb.tile([64, 512], F32, tag="bca", bufs=2, name="bca")
        nc.gpsimd.partition_broadcast(bca, rec0)
        # heads are paired on 128 partitions for a full-contract projection;
        # odd heads land at partitions 64-127 via a local SBUF DMA (DVE
        # can't write off its operand partition base).
        if j % 2 == 0:
            ocp = sb.tile([128, 512], BF16, tag="oc", bufs=8, name=f"oc{qc}_{j}")
            ocs[(qc, j // 2)] = ocp
            nc.vector.tensor_mul(ocp[0:64, :], po[0:64, :], bca)
        else:
            oct = sb.tile([64, 512], BF16, tag="oct", bufs=2, name="oct")
            nc.vector.tensor_mul(oct, po[0:64, :], bca)
            # last chunk's moves gate the tail projection: keep them off the
            # busy sync queue
            eng = nc.scalar if qc == 3 else nc.sync
            eng.dma_start(out=ocs[(qc, j // 2)][64:128, :], in_=oct)
        if (qc, j) == (0, 0):
            dump("rec_row", rec0a)
            dump("rec0", rec0)
            dump("bca", bca)
            dump("oc00", ocs[(0, 0)][0:64, :])
            dump("qk0", qk_sb[0])
            dump("qk4", qk_sb[4])
            dump("v0", vtiles[0].rearrange("p h e -> p (h e)"))

    def proj_block(qcp, st):
        ysb = sb.tile([128, D], BF16, tag="ysb", bufs=2, name="ysb")
        np_ = NH // 2
        for dmc in range(2):
            py = pp.tile([128, 512], F32, tag="po", bufs=2, name="py")
            for pr_ in range(np_):
                nc.tensor.matmul(
                    py, lhsT=ocs[(qcp, pr_)][:, st * 128:(st + 1) * 128],
                    rhs=wout_sb[pr_][:, dmc * 512:(dmc + 1) * 512],
                    start=(pr_ == 0), stop=(pr_ == np_ - 1))
            # tail projection: ACT is idle after the last exp, DVE is not
            if qcp == 3:
                nc.scalar.copy(ysb[:, dmc * 512:(dmc + 1) * 512], py)
            else:
                nc.vector.tensor_copy(ysb[:, dmc * 512:(dmc + 1) * 512], py)
        q0 = qcp * 512
        nc.sync.dma_start(out=y[q0 + st * 128:q0 + (st + 1) * 128, :], in_=ysb)

    L = 4
    n = len(units)
    for i in range(n + L):
        if i < n:
            for f in fillers.get(i, []):
                f()
            emit_S(units[i])
        ip = i - L
        if ip >= 0:
            u = units[ip]
            emit_PV(u)
            if u.get('last'):
                qc, j = u['qc'], u['j']
                normalize(qc, j)
                if qc <= 1 and j % 2 == 1 and rope_fins:
                    # finish one deferred half-1 RoPE tile per odd head-end
                    rope_fins.pop(0)()
                if qc >= 1 and j % 2 == 1:
                    proj_block(qc - 1, (j - 1) // 2)
    for st in range(4):
        proj_block(3, st)


# ======================= host-side sharding =============================

def _perm64():
    p = np.zeros(64, dtype=np.int64)
    for r in range(64):
        b, rem = divmod(r, 32)
        half, i = divmod(rem, 16)
        p[r] = 2 * (16 * b + i) + half
    return p


def _invf_sgn():
    f = np.zeros(128, dtype=np.int64)
    sg = np.zeros(128, dtype=np.float32)
    for p in range(128):
        r = p % 64
        f[p] = 16 * (r // 32) + (r % 16)
        sg[p] = -1.0 if (r % 32) < 16 else 1.0
    inv = (1.0 / THETA ** (2.0 * f / 64.0)).astype(np.float32)
    return inv.reshape(128, 1), sg.reshape(128, 1)


def make_in_maps(x, token_positions, w_qkv, w_out):
    BF = ml_dtypes.bfloat16
    x = np.asarray(x, dtype=np.float32)
    w_qkv = np.asarray(w_qkv, dtype=np.float32)
    w_out = np.asarray(w_out, dtype=np.float32)
    pos = np.asarray(token_positions)

    pm = _perm64()
    invf, sgn = _invf_sgn()
    posf = pos.astype(np.float32).reshape(1, S)
    shufP = np.zeros((128, 128), np.float32)
    for p in range(128):
        shufP[p, (p // 32) * 32 + (p % 32 + 16) % 32] = 1.0
    shufP = shufP.astype(BF)
    woutT = np.ascontiguousarray(w_out.T)

    xTs = [np.ascontiguousarray(x[b].T.astype(BF)) for b in range(4)]
    in_maps = []
    for c in range(8):
        b, g = c // 2, c % 2
        wq = w_qkv[g * EL:(g + 1) * EL]
        wk = w_qkv[D + g * EL:D + (g + 1) * EL]
        qrows = np.concatenate([wq[j * 64 + pm] for j in range(NH)], 0)
        krows = np.concatenate([wk[j * 64 + pm] for j in range(NH)], 0)
        wqkT = np.ascontiguousarray(np.concatenate([qrows, krows], 0).T.astype(BF))
        wvT = np.ascontiguousarray(
            w_qkv[2 * D + g * EL:2 * D + (g + 1) * EL].T.astype(BF))
        wout_c = np.ascontiguousarray(woutT[g * EL:(g + 1) * EL, :].astype(BF))
        in_maps.append(dict(xT=xTs[b], wqkT=wqkT, wvT=wvT, wout=wout_c,
                            posf=posf, invf=invf, sgn=sgn, shufP=shufP))
    return in_maps


def combine_outputs(results):
    """results: list of 8 dicts with 'y' [2048, 1024] bf16 -> [4, 2048, 1024]."""
    y = np.zeros((4, S, D), np.float32)
    for b in range(4):
        y[b] = (results[2 * b]["y"].astype(np.float32)
                + results[2 * b + 1]["y"].astype(np.float32))
    return y


def kernel(x, token_positions, w_qkv, w_out):
    from concourse.bass_utils import run_bass_kernel_spmd
    nc = build_nc()
    in_maps = make_in_maps(x, token_positions, w_qkv, w_out)
    res = run_bass_kernel_spmd(nc, in_maps, core_ids=list(range(8)))
    return combine_outputs(res.results)



# revision 2
# speedup vs baseline: 1.0735x; 1.0735x over previous
"""Trainium2 Bass kernel for causal multi-head self-attention with RoPE.

Problem: x[4,2048,1024], 16 heads, head_dim 64, causal, RoPE theta=1e4,
qkv proj + out proj.  Sharded over 8 cores: core c -> batch c//2, head
group c%2 (8 heads).  Host sums the two head-group partial outputs per
batch (the w_out all-reduce).

v3 design (vs v2): head-PAIR attention units so the two 64-contract
score matmuls land on PE row-tiles T0/T8 back-to-back and overlap
(2x score throughput), PV emitted BEFORE the next S so ready work is
never stuck behind an ACT-blocked S matmul (PE p-state stays warm),
shuffle matmuls decoupled from the qkv-group PSUM rotation, and a
per-pair (not per-head) softmax-denominator reciprocal chain.
"""
import numpy as np
import ml_dtypes

import concourse.bass as bass
import concourse.bacc as bacc
import concourse.mybir as mybir
import concourse.tile as tile

F32 = mybir.dt.float32
BF16 = mybir.dt.bfloat16
AF = mybir.ActivationFunctionType

THETA = 10000.0
S = 2048
D = 1024
NH = 8          # heads per core
DH = 64
EL = 512        # local head dims (NH*DH)
HALF = 1024     # tokens per QKV phase-1 half
PI_2 = 1.5707963267948966


def build_nc():
    nc = bacc.Bacc("TRN2", target_bir_lowering=False, debug=False)

    xT = nc.dram_tensor("xT", [D, S], BF16, kind="ExternalInput").ap()
    wqkT = nc.dram_tensor("wqkT", [D, 2 * EL], BF16, kind="ExternalInput").ap()
    wvT = nc.dram_tensor("wvT", [D, EL], BF16, kind="ExternalInput").ap()
    wout = nc.dram_tensor("wout", [EL, D], BF16, kind="ExternalInput").ap()
    posf = nc.dram_tensor("posf", [1, S], F32, kind="ExternalInput").ap()
    invf = nc.dram_tensor("invf", [128, 1], F32, kind="ExternalInput").ap()
    sgn = nc.dram_tensor("sgn", [128, 1], F32, kind="ExternalInput").ap()
    shufP = nc.dram_tensor("shufP", [128, 128], BF16, kind="ExternalInput").ap()
    y = nc.dram_tensor("y", [S, D], BF16, kind="ExternalOutput").ap()

    with tile.TileContext(nc) as tc:
        kernel_body(tc, xT, wqkT, wvT, wout, posf, invf, sgn, shufP, y)
    nc.compile()
    return nc


def kernel_body(tc, xT, wqkT, wvT, wout, posf, invf, sgn, shufP, y):
    with (
        tc.tile_pool(name="sb", bufs=1) as sb,
        tc.tile_pool(name="pp", bufs=1, space="PSUM") as pp,
    ):
        _kernel(tc, sb, pp, xT, wqkT, wvT, wout, posf, invf, sgn, shufP, y)


def _kernel(tc, sb, pp, xT, wqkT, wvT, wout, posf, invf, sgn, shufP, y):
    nc = tc.nc

    # ---- persistent tiles ----------------------------------------------
    qk_sb = [sb.tile([128, S], BF16, tag="qk", bufs=8, name=f"qk{t}")
             for t in range(8)]
    vtiles = [sb.tile([128, NH, 65], BF16, tag="v", bufs=16, name=f"v{t}")
              for t in range(16)]
    mtri2 = sb.tile([128, 2, 128], BF16, tag="mtri", bufs=1, name="mtri2")
    ctab = sb.tile([128, S], BF16, tag="cs", bufs=2, name="ctab")
    stab = sb.tile([128, S], BF16, tag="cs", bufs=2, name="stab")

    # ---- input DMAs (ordered for earliest PE start) --------------------
    xh = {}
    for H in (0, 1):
        xh[H] = [sb.tile([128, HALF], BF16, tag="xh", bufs=12, name=f"x{H}{d}")
                 for d in range(8)]

    def dma_xh(H):
        # half 0: 512-col chunks on two queues so the first sc-sweep can
        # start after ~128KB; half 1: full-width (fewer queue issues)
        s0 = H * HALF
        if H == 0:
            for sc in range(2):
                for d in range(8):
                    eng = nc.sync if d % 2 == 0 else nc.gpsimd
                    c0 = s0 + sc * 512
                    eng.dma_start(out=xh[H][d][:, sc * 512:(sc + 1) * 512],
                                  in_=xT[d * 128:(d + 1) * 128, c0:c0 + 512])
        else:
            for d in range(8):
                eng = nc.sync if d % 2 == 0 else nc.gpsimd
                eng.dma_start(out=xh[H][d],
                              in_=xT[d * 128:(d + 1) * 128, s0:s0 + HALF])

    def dma_wqk(blk, ep, eng=None):
        wcol = blk * EL + ep * 256
        wts = []
        for d in range(8):
            w_t = sb.tile([128, 256], BF16, tag="w", bufs=12, name="wqk")
            (eng or nc.scalar).dma_start(
                out=w_t, in_=wqkT[d * 128:(d + 1) * 128, wcol:wcol + 256])
            wts.append(w_t)
        return wts

    wts00 = dma_wqk(0, 0)   # first q/k weight group before x: PE starts sooner
    wts01 = dma_wqk(0, 1)   # second group before wv (needed ~8us earlier)
    dma_xh(0)

    # small table loads AFTER the critical x stream (~650ns queue cost each;
    # tables aren't consumed until ~10us in)
    invf_sb = sb.tile([128, 1], F32, tag="cvec", bufs=2)
    nc.sync.dma_start(out=invf_sb, in_=invf)
    sgn_sb = sb.tile([128, 1], F32, tag="cvec", bufs=2)
    nc.sync.dma_start(out=sgn_sb, in_=sgn)
    shufP_sb = sb.tile([128, 128], BF16, tag="shufP", bufs=1)
    nc.sync.dma_start(out=shufP_sb, in_=shufP)
    posf_sb = sb.tile([1, S], F32, tag="rt", bufs=2)
    nc.sync.dma_start(out=posf_sb, in_=posf)

    wv_sb = []
    for d in range(8):
        wv_t = sb.tile([128, EL], BF16, tag="wv", bufs=8, name=f"wv{d}")
        nc.scalar.dma_start(out=wv_t, in_=wvT[d * 128:(d + 1) * 128, :])
        wv_sb.append(wv_t)

    # ---- phase 0: RoPE tables + causal triangle mask + v ones ----------
    nc.vector.memset(mtri2, 1.0)
    for b in range(2):
        nc.gpsimd.affine_select(out=mtri2[:, b, :], in_=mtri2[:, b, :],
                                compare_op=mybir.AluOpType.is_ge, fill=0.0,
                                base=0, channel_multiplier=-1, pattern=[[1, 128]])
    for t in range(16):
        # softmax-denominator ones column (col 64 of each head's v tile)
        nc.gpsimd.memset(vtiles[t][:, :, 64:65], 1.0)

    pos_b = sb.tile([128, S], F32, tag="rt", bufs=2)
    nc.gpsimd.partition_broadcast(pos_b, posf_sb)
    angles = sb.tile([128, S], F32, tag="rt", bufs=2)
    nc.vector.tensor_scalar_mul(angles, pos_b, invf_sb)
    # range-reduce angles into [-pi, pi]:  k = round(angle / 2pi) via the
    # magic-constant trick, then 3-term Cody-Waite  x - k*2pi.
    TWO_PI = 6.283185307179586
    MAGIC = 1.5 * 2.0 ** 23
    kq = sb.tile([128, S], F32, tag="rt", bufs=2)
    nc.vector.tensor_scalar_mul(kq, angles, 1.0 / TWO_PI)
    nc.vector.tensor_scalar(kq, kq, MAGIC, MAGIC,
                            mybir.AluOpType.add, mybir.AluOpType.subtract)
    CW1 = 6.28125
    CW2 = float(np.float32(TWO_PI - CW1))
    CW3 = float(TWO_PI - CW1 - np.float64(np.float32(TWO_PI - CW1)))
    nc.vector.cody_waite_cascade(angles, angles, kq, CW1, CW2, CW3)
    nc.vector.add_range_wrap(kq, angles, 0.0, np.pi, TWO_PI)
    nc.scalar.activation(stab, kq, AF.Sin)
    nc.vector.add_range_wrap(angles, angles, PI_2, np.pi, TWO_PI)
    nc.scalar.activation(ctab, angles, AF.Sin)
    nc.vector.tensor_scalar_mul(stab, stab, sgn_sb)

    # ---- phase 1 building blocks ---------------------------------------
    def qk_ei(H, blk, ep, ei, wts):
        """16 qkv matmuls for one 128-row output block + PSUM->SBUF copy.
        Returns a closure that does the RoPE-partner shuffle (own PSUM
        alloc, decoupled from this group's rotation slot)."""
        s0 = H * HALF
        ps = pp.tile([128, 2, 512], F32, tag="ps", bufs=2, name="psqk")
        # sc-major then d-major: each matmul only needs one 512-col half of
        # one xh tile, so the PE streams right behind the x DMA
        for sc in range(2):
            for d in range(8):
                nc.tensor.matmul(
                    ps[:, sc, :], lhsT=wts[d][:, ei * 128:(ei + 1) * 128],
                    rhs=xh[H][d][:, sc * 512:(sc + 1) * 512],
                    start=(d == 0), stop=(d == 7))
        t = blk * 4 + ep * 2 + ei
        sl = qk_sb[t][:, s0:s0 + HALF]
        # alternate the PSUM->SBUF copy between ACT and DVE to balance queues
        if ei == 0:
            nc.scalar.copy(sl, ps.rearrange("p a b -> p (a b)"))
        else:
            nc.vector.tensor_copy(sl, ps.rearrange("p a b -> p (a b)"))

        def shuf(sl=sl, s0=s0):
            # RoPE partner values via a PE permutation matmul reading the
            # already-copied sl; allocates its own ps slot so the qkv
            # group's slot is free as soon as the sl copy drains.
            ps2 = pp.tile([128, 2, 512], F32, tag="ps", bufs=2, name="psshuf")
            shufB = sb.tile([128, HALF], BF16, tag="rsh", bufs=2, name="shufB")
            for sc in range(2):
                nc.tensor.matmul(ps2[:, sc, :], lhsT=shufP_sb,
                                 rhs=sl[:, sc * 512:(sc + 1) * 512],
                                 start=True, stop=True)
            if ei == 0:
                nc.vector.tensor_copy(shufB, ps2.rearrange("p a b -> p (a b)"))
            else:
                nc.scalar.copy(shufB, ps2.rearrange("p a b -> p (a b)"))

            def fin(sl=sl, shufB=shufB, s0=s0):
                t1 = sb.tile([128, HALF], BF16, tag="rt1", bufs=2, name="t1")
                nc.vector.tensor_mul(t1, sl, ctab[:, s0:s0 + HALF])
                nc.vector.tensor_mul(shufB, shufB, stab[:, s0:s0 + HALF])
                nc.vector.tensor_add(sl, t1, shufB)
            return fin
        return shuf

    def v_pair(H, p):
        # half-0 uses the attention-idle po tag so the ps rotation serves
        # only the qkv chains
        if H == 0:
            pss = [pp.tile([128, 512], F32, tag="po", bufs=4, name="psv")
                   for _ in range(2)]
        else:
            ps = pp.tile([128, 2, 512], F32, tag="ps", bufs=2, name="psv")
            pss = [ps[:, 0, :], ps[:, 1, :]]
        for d in range(8):
            for i in range(2):
                st = 2 * p + i
                nc.tensor.matmul(
                    pss[i], lhsT=xh[H][d][:, st * 128:(st + 1) * 128],
                    rhs=wv_sb[d], start=(d == 0), stop=(d == 7))
        for i in range(2):
            st = 2 * p + i
            vt = vtiles[H * 8 + st]
            if i == 0:
                nc.scalar.copy(vt[:, :, 0:64],
                               pss[i].rearrange("p (h e) -> p h e", h=NH))
            else:
                nc.vector.tensor_copy(vt[:, :, 0:64],
                                      pss[i].rearrange("p (h e) -> p h e", h=NH))

    # ---- phase 1, half 0 -----------------------------------------------
    pre = {(0, 0): wts00, (0, 1): wts01}
    for blk in range(2):
        for ep in range(2):
            wts = pre.get((blk, ep)) or dma_wqk(blk, ep)
            shufs = [qk_ei(0, blk, ep, ei, wts) for ei in range(2)]
            v_pair(0, blk * 2 + ep)
            for sh in shufs:
                sh()()   # shuffle matmuls + copy, then RoPE fin inline

    dma_xh(1)
    wout_sb = []
    for pair in range(NH // 2):
        wo_t = sb.tile([128, D], BF16, tag="wout", bufs=4, name=f"wo{pair}")
        nc.scalar.dma_start(out=wo_t, in_=wout[pair * 128:(pair + 1) * 128, :])
        wout_sb.append(wo_t)

    # ---- phase 2: head-pair attention units, global software pipeline --
    # Unit = one 128-key tile for one head pair: the two 64-contract score
    # matmuls (even head -> PE row tile T0, odd head -> T8) are adjacent
    # and independent so the array halves overlap.  Diag tiles first (max
    # mask slack), then mask-free full tiles.
    units = []
    for qc in range(4):
        nd = 4 * qc
        for p in range(4):
            kis = list(range(nd, nd + 4)) + list(range(0, nd))
            for idx, ki in enumerate(kis):
                units.append(dict(
                    qc=qc, p=p, ki=ki, rel=max(0, ki * 128 - qc * 512),
                    first=(idx == 0), last=(idx == len(kis) - 1),
                    diag=(ki >= nd)))
    n = len(units)

    # fillers before S-emission of unit index i
    fillers = {}
    rope_fins = []

    def run_qk1(blk, ep, ei, wts):
        shuf = qk_ei(1, blk, ep, ei, wts)

        def run_shuf(shuf=shuf):
            rope_fins.append(shuf())
        return run_shuf

    seq = []
    wts1 = {}
    for blk in range(2):
        for ep in range(2):
            def load(blk=blk, ep=ep):
                wts1[(blk, ep)] = dma_wqk(blk, ep)
            shuf_runs = []

            def ei0(blk=blk, ep=ep, shuf_runs=shuf_runs):
                shuf_runs.append(run_qk1(blk, ep, 0, wts1[(blk, ep)]))

            def ei1(blk=blk, ep=ep, shuf_runs=shuf_runs):
                shuf_runs.append(run_qk1(blk, ep, 1, wts1[(blk, ep)]))

            def shufs(shuf_runs=shuf_runs):
                for sr in shuf_runs:
                    sr()
            seq.append([load, ei0])
            seq.append([ei1])
            seq.append([lambda p=blk * 2 + ep: v_pair(1, p)])
            seq.append([shufs])
    for i, fs in enumerate(seq):
        fillers.setdefault(2 * i, []).extend(fs)

    po_map = {}
    ocs = {}

    def emit_S(u):
        qc, p, ki, rel = u['qc'], u['p'], u['ki'], u['rel']
        q0 = qc * 512
        qt = qk_sb[p]
        kt = qk_sb[4 + p]
        ps = pp.tile([128, 2, 512], F32, tag="ps", bufs=2, name="ps")
        pr = sb.tile([128, 2, 512], BF16, tag="pr", bufs=8, name="pr")
        u['pr'] = pr
        for par in range(2):
            jb = par * 64
            nc.tensor.matmul(
                ps[:, par, rel:512],
                lhsT=kt[jb:jb + 64, ki * 128:(ki + 1) * 128],
                rhs=qt[jb:jb + 64, q0 + rel:q0 + 512], start=True, stop=True)
        nc.scalar.activation(pr[:, :, rel:512], ps[:, :, rel:512],
                             AF.Exp, scale=0.125)
        if u['diag']:
            nc.vector.tensor_mul(pr[:, :, rel:rel + 128],
                                 pr[:, :, rel:rel + 128], mtri2)

    def emit_PV(u):
        qc, p, ki, rel = u['qc'], u['p'], u['ki'], u['rel']
        key = (qc, p)
        if key not in po_map:
            po_map[key] = (pp.tile([65, 512], F32, tag="po", bufs=4, name="poe"),
                           pp.tile([65, 512], F32, tag="po", bufs=4, name="poo"))
        poe, poo = po_map[key]
        pr = u['pr']
        nc.tensor.matmul(poe[:, rel:512], lhsT=vtiles[ki][:, 2 * p, :],
                         rhs=pr[:, 0, rel:512], start=u['first'], stop=u['last'])
        nc.tensor.matmul(poo[:, rel:512], lhsT=vtiles[ki][:, 2 * p + 1, :],
                         rhs=pr[:, 1, rel:512], start=u['first'], stop=u['last'])

    def normalize(qc, p):
        poe, poo = po_map.pop((qc, p))
        # custom-DVE ops read zeros from PSUM and misbehave off partition 0:
        # stage both dens PSUM->SBUF (aligned), DMA to partition 0, recip
        # there, one gpsimd broadcast for the pair.
        den = sb.tile([65, 2, 512], F32, tag="rec", bufs=2, name="den")
        nc.vector.tensor_copy(den[64:65, 0, :], poe[64:65, :])
        nc.vector.tensor_copy(den[64:65, 1, :], poo[64:65, :])
        rec0a = sb.tile([1, 2, 512], F32, tag="rec0a", bufs=2, name="rec0a")
        nc.sync.dma_start(out=rec0a, in_=den[64:65, :, :])
        rec0 = sb.tile([1, 2, 512], F32, tag="rec0", bufs=2, name="rec0")
        nc.vector.reciprocal_approx_fast(rec0, rec0a)
        bca = sb.tile([64, 2, 512], F32, tag="bca", bufs=2, name="bca")
        nc.gpsimd.partition_broadcast(bca, rec0)
        # heads are paired on 128 partitions for a full-contract projection;
        # odd head lands at partitions 64-127 via a local SBUF DMA (DVE
        # can't write off its operand partition base).
        ocp = sb.tile([128, 512], BF16, tag="oc", bufs=8, name=f"oc{qc}_{p}")
        ocs[(qc, p)] = ocp
        nc.vector.tensor_mul(ocp[0:64, :], poe[0:64, :], bca[:, 0, :])
        oct = sb.tile([64, 512], BF16, tag="oct", bufs=2, name="oct")
        nc.vector.tensor_mul(oct, poo[0:64, :], bca[:, 1, :])
        # last chunk's moves gate the tail projection: keep them off the
        # busy sync queue
        eng = nc.scalar if qc == 3 else nc.sync
        eng.dma_start(out=ocp[64:128, :], in_=oct)

    def proj_block(qcp, st, tail=False):
        ysb = sb.tile([128, D], BF16, tag="ysb", bufs=2, name="ysb")
        np_ = NH // 2
        for dmc in range(2):
            py = pp.tile([128, 512], F32, tag="po", bufs=4, name="py")
            for pr_ in range(np_):
                nc.tensor.matmul(
                    py, lhsT=ocs[(qcp, pr_)][:, st * 128:(st + 1) * 128],
                    rhs=wout_sb[pr_][:, dmc * 512:(dmc + 1) * 512],
                    start=(pr_ == 0), stop=(pr_ == np_ - 1))
            # tail: both ACT and DVE are free; split so the last copies
            # drain in parallel instead of serializing on one engine
            if tail:
                eng = nc.scalar if dmc == 0 else nc.vector
            else:
                eng = nc.vector
            if eng is nc.scalar:
                nc.scalar.copy(ysb[:, dmc * 512:(dmc + 1) * 512], py)
            else:
                nc.vector.tensor_copy(ysb[:, dmc * 512:(dmc + 1) * 512], py)
        q0 = qcp * 512
        nc.sync.dma_start(out=y[q0 + st * 128:q0 + (st + 1) * 128, :], in_=ysb)

    L = 4
    for i in range(n + L):
        # PV first: its pr is ready, so the PE never idles behind an S
        # matmul that's still waiting on the exp of an earlier unit.
        ip = i - L
        if ip >= 0:
            u = units[ip]
            emit_PV(u)
            if u['last']:
                qc, p = u['qc'], u['p']
                normalize(qc, p)
                if rope_fins:
                    # finish one deferred half-1 RoPE tile per pair-end
                    rope_fins.pop(0)()
                if qc >= 1:
                    proj_block(qc - 1, p)
        if i < n:
            for f in fillers.get(i, []):
                f()
            emit_S(units[i])
    for st in range(4):
        proj_block(3, st, tail=True)


# ======================= host-side sharding =============================

def _perm64():
    p = np.zeros(64, dtype=np.int64)
    for r in range(64):
        b, rem = divmod(r, 32)
        half, i = divmod(rem, 16)
        p[r] = 2 * (16 * b + i) + half
    return p


def _invf_sgn():
    f = np.zeros(128, dtype=np.int64)
    sg = np.zeros(128, dtype=np.float32)
    for p in range(128):
        r = p % 64
        f[p] = 16 * (r // 32) + (r % 16)
        sg[p] = -1.0 if (r % 32) < 16 else 1.0
    inv = (1.0 / THETA ** (2.0 * f / 64.0)).astype(np.float32)
    return inv.reshape(128, 1), sg.reshape(128, 1)


def make_in_maps(x, token_positions, w_qkv, w_out):
    BF = ml_dtypes.bfloat16
    x = np.asarray(x, dtype=np.float32)
    w_qkv = np.asarray(w_qkv, dtype=np.float32)
    w_out = np.asarray(w_out, dtype=np.float32)
    pos = np.asarray(token_positions)

    pm = _perm64()
    invf, sgn = _invf_sgn()
    posf = pos.astype(np.float32).reshape(1, S)
    shufP = np.zeros((128, 128), np.float32)
    for p in range(128):
        shufP[p, (p // 32) * 32 + (p % 32 + 16) % 32] = 1.0
    shufP = shufP.astype(BF)
    woutT = np.ascontiguousarray(w_out.T)

    xTs = [np.ascontiguousarray(x[b].T.astype(BF)) for b in range(4)]
    in_maps = []
    for c in range(8):
        b, g = c // 2, c % 2
        wq = w_qkv[g * EL:(g + 1) * EL]
        wk = w_qkv[D + g * EL:D + (g + 1) * EL]
        qrows = np.concatenate([wq[j * 64 + pm] for j in range(NH)], 0)
        krows = np.concatenate([wk[j * 64 + pm] for j in range(NH)], 0)
        wqkT = np.ascontiguousarray(np.concatenate([qrows, krows], 0).T.astype(BF))
        wvT = np.ascontiguousarray(
            w_qkv[2 * D + g * EL:2 * D + (g + 1) * EL].T.astype(BF))
        wout_c = np.ascontiguousarray(woutT[g * EL:(g + 1) * EL, :].astype(BF))
        in_maps.append(dict(xT=xTs[b], wqkT=wqkT, wvT=wvT, wout=wout_c,
                            posf=posf, invf=invf, sgn=sgn, shufP=shufP))
    return in_maps


def combine_outputs(results):
    """results: list of 8 dicts with 'y' [2048, 1024] bf16 -> [4, 2048, 1024]."""
    y = np.zeros((4, S, D), np.float32)
    for b in range(4):
        y[b] = (results[2 * b]["y"].astype(np.float32)
                + results[2 * b + 1]["y"].astype(np.float32))
    return y


def kernel(x, token_positions, w_qkv, w_out):
    from concourse.bass_utils import run_bass_kernel_spmd
    nc = build_nc()
    in_maps = make_in_maps(x, token_positions, w_qkv, w_out)
    res = run_bass_kernel_spmd(nc, in_maps, core_ids=list(range(8)))
    return combine_outputs(res.results)
